# revision 25
# baseline (speedup 1.0000x reference)
"""CARC attention processor kernel for 8 Trainium2 NeuronCores.

Reference computation (B=1, L=4096, C=640, H=10, D=64):
    q/k/v = hidden @ Wq/Wk/Wv, split into 10 heads of 64
    k_cat = [k, 0.42*K_bg], v_cat = [v, 0.42*V_bg]   (key length 8192)
    out   = softmax(q k_cat^T / 8) v_cat, heads merged, @ Wo + bo

Sharding: queries are split 512 per core; every core computes all 10 heads
for its queries (k/v projections replicated per core).  Output is a
disjoint row-slice per core; the host concatenates.

All tensors are pre-converted to bf16 and pre-laid-out on the host, so DMA
lands directly in the SBUF layouts the matmuls consume (no staging casts).

Softmax skips max-subtraction (scores are ~N(0,1)).  The exp work is split
between ScalarE (hardware Exp LUT) and VectorE (Schraudolph fast-exp2:
one tensor_scalar computing round(a*s + b) into int16, bitcast to bf16),
alternating key tiles, so neither engine is the bottleneck.  The softmax
denominator comes from a ones-column appended to V in the probs@V matmul;
the output-projection bias is folded in as a 65th row of Wo.

Heads are processed in pairs (head A on partitions 0-63, head B on 64-127);
the two QK^T matmuls of a pair co-stream through disjoint PE row groups
via tile_position.
"""

import numpy as np

import concourse.bass as bass
import concourse.mybir as mybir
import concourse.tile as tile

F32 = mybir.dt.float32
BF16 = mybir.dt.bfloat16
I16 = mybir.dt.int16
I8 = mybir.dt.int8
E4 = mybir.dt.float8e4
E5 = mybir.dt.float8e5
AF = mybir.ActivationFunctionType
ALU = mybir.AluOpType

# Problem constants (hardcoded per contract)
B, L, C = 1, 4096, 640
H, D = 10, 64
ALPHA = 0.42
N_CORES = 8
SCALE = 1.0 / np.sqrt(D)  # 0.125
LOG2E = 1.4426950408889634
# fast-exp2 magic: i16 = rint(FEXP_A*escale*s + FEXP_B); bitcast bf16 ~ exp(escale*s)
# (DVE converts fp32->i16 round-half-even; c=-5.6 calibrated, max rel err 3.3%)
FEXP_B = 127.0 * 128.0 - 5.6
# e5m2 variant for fp8 probs: i8 = rint(4*log2(e)*escale*s + FEXP5_B); bits ~ e5m2
FEXP5_B = 15.0 * 4.0 - 1.4


class Cfg:
    def __init__(self, H=H, C=C, Lk=L, Q=L // N_CORES):
        assert C % 128 == 0 and Lk % 1024 == 0 and Q % 128 == 0 and Q <= 512
        assert H % 2 == 0
        self.H, self.C, self.Lk, self.Q = H, C, Lk, Q
        self.n_cc = C // 128      # contraction chunks for projections
        self.n_kt = Lk // 128     # key tiles per source (self / bg)
        self.n_qt = Q // 128      # query tiles of this core


def emit(nc: bass.Bass, cfg: Cfg):
    Hh, Cc, Lk, Q = cfg.H, cfg.C, cfg.Lk, cfg.Q
    n_cc, n_kt, n_qt = cfg.n_cc, cfg.n_kt, cfg.n_qt
    n_pair = Hh // 2

    hqT = nc.declare_dram_parameter("hqTb", [128, n_cc, Q], BF16, isOutput=False)
    kbg = nc.declare_dram_parameter("Kbgb", [n_pair, 128, Lk], BF16, isOutput=False)
    vbg = nc.declare_dram_parameter(
        "Vbgb", [n_pair, 128, 8, 4 * 2 * (D + 1)], BF16, isOutput=False
    )
    wq = nc.declare_dram_parameter("Wqb", [128, n_cc, Cc], BF16, isOutput=False)
    wk = nc.declare_dram_parameter("Wkb", [128, n_cc, Cc], BF16, isOutput=False)
    wv = nc.declare_dram_parameter("Wvb", [128, n_cc, Cc], BF16, isOutput=False)
    wob = nc.declare_dram_parameter("WoBb", [D + 1, Hh, Cc], BF16, isOutput=False)
    out = nc.declare_dram_parameter("out", [Q, Cc], F32, isOutput=True)

    with tile.TileContext(nc) as tc:
        with (
            tc.tile_pool(name="singles", bufs=1) as singles,
            tc.tile_pool(name="kv", bufs=2) as kv,
            tc.tile_pool(name="probs", bufs=4) as probs_pool,
            tc.tile_pool(name="outsb", bufs=2) as outsb_pool,
            tc.tile_pool(name="fin", bufs=2) as fin_pool,
            tc.tile_pool(name="dram", bufs=1, space="DRAM") as dram,
            tc.tile_pool(name="ps_a", bufs=2, space="PSUM") as ps_a,
            tc.tile_pool(name="ps_sc", bufs=2, space="PSUM") as ps_sc,
            tc.tile_pool(name="ps_ctx", bufs=1, space="PSUM") as ps_ctx,
        ):
            # ---- persistent SBUF tensors, DMA'd directly in final layout ----
            hq_bf = singles.tile([128, n_cc, Q], BF16, tag="hq_bf")
            wq_bf = singles.tile([128, n_cc, Cc], BF16, tag="wq_bf")
            wk_bf = singles.tile([128, n_cc, Cc], BF16, tag="wk_bf")
            wv_bf = singles.tile([128, n_cc, Cc], BF16, tag="wv_bf")
            wob_bf = singles.tile([D + 1, Hh, Cc], BF16, tag="wob_bf")
            qT2_all = singles.tile([128, n_pair, Q], BF16, tag="qT2_all")
            ctxT_all = singles.tile([D + 1, Hh, Q], BF16, tag="ctxT_all")
            ones64 = singles.tile([D + 1, D], F32, tag="ones64")
            nc.vector.memset(ones64, 1.0)
            for h in range(Hh):
                nc.vector.memset(ctxT_all[D : D + 1, h, :], 1.0)

            nc.sync.dma_start(out=hq_bf, in_=hqT[:, :, :])
            nc.sync.dma_start(out=wq_bf, in_=wq[:, :, :])
            nc.sync.dma_start(out=wk_bf, in_=wk[:, :, :])
            nc.sync.dma_start(out=wv_bf, in_=wv[:, :, :])

            # ---- q projections, head pairs packed on partitions ----
            for p in range(n_pair):
                ps = ps_a.tile([128, 512], F32, tag="ps_a", name=f"qps{p}")
                for i in range(n_cc):
                    nc.tensor.matmul(
                        ps[:, 0:Q],
                        lhsT=wq_bf[:, i, 128 * p : 128 * (p + 1)],
                        rhs=hq_bf[:, i, :],
                        start=(i == 0),
                        stop=(i == n_cc - 1),
                    )
                nc.scalar.copy(out=qT2_all[:, p, :], in_=ps[:, 0:Q])

            # ---- k/v projections, sharded across cores ----
            # Each core projects only ITS 512-key slice (which equals its
            # query slice hq) for all pairs, then a per-pair AllGather
            # rebuilds the full 4096-key k/v on every core.  This removes the
            # 8x-replicated projection matmuls from the PE.
            # Slice layout per pair: [512 kT cols | 4*(2*(D+1)) v cols].
            VW = 2 * (D + 1)          # 130: v cols per key tile (both heads)
            SLW = 512 + 4 * VW        # 1032
            ks_sl = singles.tile([128, n_pair, 512], BF16, tag="ks_sl")
            vs_sl = singles.tile([128, n_pair, 4, 2, D + 1], BF16, tag="vs_sl")
            for p in range(n_pair):
                ps = ps_a.tile([128, 512], F32, tag="ps_a", name=f"skp{p}")
                for i in range(n_cc):
                    nc.tensor.matmul(
                        ps,
                        lhsT=wk_bf[:, i, 128 * p : 128 * (p + 1)],
                        rhs=hq_bf[:, i, :],
                        start=(i == 0),
                        stop=(i == n_cc - 1),
                    )
                nc.scalar.copy(out=ks_sl[:, p, :], in_=ps)
                ps2 = ps_a.tile([128, 4, 128], F32, tag="ps_a", name=f"svp{p}")
                for j in range(4):
                    for i in range(n_cc):
                        nc.tensor.matmul(
                            ps2[:, j, :],
                            lhsT=hq_bf[:, i, 128 * j : 128 * (j + 1)],
                            rhs=wv_bf[:, i, 128 * p : 128 * (p + 1)],
                            start=(i == 0),
                            stop=(i == n_cc - 1),
                        )
                for hi in range(2):
                    nc.scalar.copy(
                        out=vs_sl[:, p, :, hi, 0:D],
                        in_=ps2[:, :, D * hi : D * (hi + 1)],
                    )
                    nc.vector.memset(vs_sl[:, p, :, hi, D : D + 1], 1.0)

            # bounce + AllGather per pair, emitted back-to-back so CC(0)
            # triggers as soon as pair 0's slice is staged (the gpsimd queue
            # is serial; all-bounces-first would delay the first gather)
            cc_out = []
            for p in range(n_pair):
                bnc = dram.tile([128, SLW], BF16, tag=f"ccin{p}")
                ago = dram.tile([8 * 128, SLW], BF16, tag=f"ccout{p}")
                nc.gpsimd.dma_start(bnc[:, 0:512], ks_sl[:, p, :])
                nc.gpsimd.dma_start(
                    bnc[:, 512:SLW],
                    vs_sl[:, p, :, :, :].rearrange("q a h d -> q (a h d)"),
                )
                nc.gpsimd.collective_compute(
                    "AllGather",
                    ALU.bypass,
                    replica_groups=[list(range(N_CORES))],
                    ins=[bnc.opt()],
                    outs=[ago.opt()],
                )
                cc_out.append(ago)
            nc.sync.dma_start(out=wob_bf, in_=wob[:, :, :])

            # ---- deferred per-pair normalization ----
            norm_queue = []

            def emit_normalize():
                while norm_queue:
                    pn, unn = norm_queue.pop(0)
                    for hi in range(2):
                        h = 2 * pn + hi
                        fin = fin_pool.tile(
                            [D + 1, Q], F32, tag="fin", name=f"fin{h}"
                        )
                        bc = ps_a.tile([128, 512], F32, tag="ps_a", name=f"bc{h}")
                        nc.tensor.matmul(
                            bc[0:D, 0:Q],
                            lhsT=ones64[D : D + 1, :],
                            rhs=unn[D : D + 1, hi, :],
                            start=True,
                            stop=True,
                            tile_position=(D, 0),
                        )
                        nc.vector.reciprocal(fin[0:D, :], bc[0:D, 0:Q])
                        nc.vector.tensor_mul(
                            ctxT_all[0:D, h, :], unn[0:D, hi, :], fin[0:D, :]
                        )

            # ---- per head-pair: AllGather k/v, load bg kv, attention ----
            for p in range(n_pair):
                kT2 = kv.tile([128, Lk], BF16, tag="kT")
                v2 = kv.tile([128, 8, 4 * VW], BF16, tag="v")
                kbg2 = kv.tile([128, Lk], BF16, tag="kbg")
                vbg2 = kv.tile([128, 8, 4 * VW], BF16, tag="vbg")

                nc.sync.dma_start(out=kbg2, in_=kbg[p, :, :])
                nc.sync.dma_start(out=vbg2, in_=vbg[p, :, :, :])

                ag = cc_out[p].opt().rearrange("(r q) n -> q r n", q=128)
                nc.sync.dma_start(
                    out=kT2.rearrange("q (r n) -> q r n", r=8),
                    in_=ag[:, :, 0:512],
                )
                nc.sync.dma_start(out=v2, in_=ag[:, :, 512:SLW])

                # normalize the PREVIOUS pair while this pair's AllGather is
                # still in flight
                emit_normalize()

                # ---- attention for the pair ----
                # ctx accumulators: head A in PSUM bank 0, head B in bank 1.
                # Software-pipelined: scores+exp for step i are issued before
                # the ctx matmuls of step i-2, so the PE never waits on exp.
                ctx2 = ps_ctx.tile([D + 1, 2, 512], F32, tag="ctx", name=f"ctx{p}")
                seq = [(src, kt) for src in range(2) for kt in range(n_kt)]

                def issue_front(idx):
                    src, kt = seq[idx]
                    kk = kT2 if src == 0 else kbg2
                    e_scale = SCALE if src == 0 else SCALE * ALPHA
                    scs = ps_sc.tile(
                        [128, 2, Q], F32, tag="sc", name=f"sc{p}{src}{kt}"
                    )
                    for hi in range(2):
                        nc.tensor.matmul(
                            scs[:, hi, :],
                            lhsT=kk[
                                D * hi : D * (hi + 1),
                                128 * kt : 128 * (kt + 1),
                            ],
                            rhs=qT2_all[D * hi : D * (hi + 1), p, :],
                            start=True,
                            stop=True,
                            tile_position=(D * hi, 0),
                        )
                    # exp alternates ScalarE (LUT) / VectorE (fast-exp2)
                    if idx % 2 == 0:
                        pr = probs_pool.tile(
                            [128, 2, Q], BF16, tag="pr", name=f"pr{p}{src}{kt}"
                        )
                        nc.scalar.activation(pr, scs, AF.Exp, scale=e_scale)
                        pr_bf = pr
                    else:
                        pri = probs_pool.tile(
                            [128, 2, Q], I16, tag="pri", name=f"pri{p}{src}{kt}"
                        )
                        nc.vector.tensor_scalar(
                            pri,
                            scs,
                            128.0 * LOG2E * e_scale,
                            FEXP_B,
                            ALU.mult,
                            ALU.add,
                        )
                        pr_bf = pri.bitcast(BF16)
                    return (v2 if src == 0 else vbg2), kt, pr_bf, idx == 0

                def issue_ctx(st, last):
                    vv, kt, pr_bf, first = st
                    for hi in range(2):
                        off = VW * (kt % 4) + (D + 1) * hi
                        nc.tensor.matmul(
                            ctx2[:, hi, 0:Q],
                            lhsT=vv[:, kt // 4, off : off + D + 1],
                            rhs=pr_bf[:, hi, :],
                            start=first,
                            stop=last,
                        )

                # 2-kt batches: [sc,sc,sc,sc] then [ctx,ctx,ctx,ctx] so
                # same-row-group LDWEIGHTS hide in the background weight
                # buffer and only two row-group boundaries are exposed per
                # batch instead of four.
                DEPTH = 4
                pipe = []
                for idx in range(0, len(seq), 2):
                    pipe.append(issue_front(idx))
                    pipe.append(issue_front(idx + 1))
                    while len(pipe) > DEPTH:
                        issue_ctx(pipe.pop(0), False)
                for i, st in enumerate(pipe):
                    issue_ctx(st, i == len(pipe) - 1)
                # Free the ctx PSUM banks immediately with one wide copy to
                # SBUF; the normalize itself is emitted deferred (after the
                # next pair's projections) so its PSUM-ring slots don't gate
                # the next pair's projection matmuls.
                unn = fin_pool.tile([D + 1, 2, Q], F32, tag="unn", name=f"unn{p}")
                nc.scalar.copy(out=unn, in_=ctx2)
                norm_queue.append((p, unn))
                if p == n_pair - 1:
                    emit_normalize()

            # ---- output projection: out[qt] = sum_h ctxT_h^T @ WoB_h ----
            for qt in range(n_qt):
                o_sb = outsb_pool.tile([128, Cc], F32, tag="o_sb")
                for n0 in range(0, Cc, 512):
                    nw = min(512, Cc - n0)
                    ps = ps_sc.tile([128, 2, Q], F32, tag="sc", name=f"ops{qt}{n0}")
                    for h in range(Hh):
                        nc.tensor.matmul(
                            ps[:, 0, 0:nw],
                            lhsT=ctxT_all[:, h, 128 * qt : 128 * (qt + 1)],
                            rhs=wob_bf[:, h, n0 : n0 + nw],
                            start=(h == 0),
                            stop=(h == Hh - 1),
                        )
                    if n0 == 0:
                        nc.scalar.copy(out=o_sb[:, n0 : n0 + nw], in_=ps[:, 0, 0:nw])
                    else:
                        nc.vector.tensor_copy(
                            out=o_sb[:, n0 : n0 + nw], in_=ps[:, 0, 0:nw]
                        )
                nc.sync.dma_start(out=out[128 * qt : 128 * (qt + 1), :], in_=o_sb)
    return nc


def split_waits(nc, limit=1):
    """This container's walrus rejects >limit sync waits per instruction;
    hoist excess waits onto standalone EventSemaphore instructions."""
    cnt = 0
    for f in nc.m.functions:
        for bb in f.blocks:
            fixed = []
            for inst in bb.instructions:
                si = inst.sync_info
                if si is not None and len(si.on_wait) > limit:
                    waits = list(si.on_wait)
                    extra, keep = waits[:-limit], waits[-limit:]
                    for w in extra:
                        cnt += 1
                        ev = mybir.InstEventSemaphore(
                            name=f"I-waitsplit-{cnt}", ins=[], outs=[]
                        )
                        ev.engine = inst.engine
                        ev.sync_info = mybir.SyncInfo(on_wait=[w], on_update=[])
                        nc.register_instruction(ev)
                        fixed.append(ev)
                    si.on_wait = keep
                fixed.append(inst)
            bb.instructions[:] = fixed
    return cnt


def build_bass(cfg: Cfg | None = None):
    cfg = cfg or Cfg()
    nc = bass.Bass()
    emit(nc, cfg)
    split_waits(nc)
    return nc


def make_in_maps(hidden_states, K_bg, V_bg, Wq, Wk, Wv, Wo, bo):
    import ml_dtypes

    bf = ml_dtypes.bfloat16
    n_cc, n_kt, n_pair = C // 128, L // 128, H // 2

    hT = np.asarray(hidden_states, np.float32)[0].T  # [C, L]
    hTb = np.ascontiguousarray(
        hT.reshape(n_cc, 128, L).transpose(1, 0, 2)
    ).astype(bf)  # sliced per core below; full copy never shipped

    def wprep(w):
        return np.ascontiguousarray(
            np.asarray(w, np.float32).reshape(n_cc, 128, C).transpose(1, 0, 2)
        ).astype(bf)

    WoB = np.zeros((H, D + 1, C), np.float32)
    WoB[:, :D, :] = np.asarray(Wo, np.float32).reshape(H, D, C)
    WoB[0, D, :] = np.asarray(bo, np.float32)
    WoBb = np.ascontiguousarray(WoB.transpose(1, 0, 2)).astype(bf)

    Kbgb = np.ascontiguousarray(
        np.asarray(K_bg, np.float32).transpose(0, 2, 1).reshape(n_pair, 128, L)
    ).astype(bf)
    Vbgb = np.ones((n_pair, 128, n_kt, 2, D + 1), np.float32)
    Vbgb[:, :, :, :, :D] = (
        (ALPHA * np.asarray(V_bg, np.float32))
        .reshape(n_pair, 2, n_kt, 128, D)
        .transpose(0, 3, 2, 1, 4)
    )
    Vbgb = Vbgb.astype(bf).reshape(n_pair, 128, 8, 4 * 2 * (D + 1))

    common = {
        "Kbgb": Kbgb,
        "Vbgb": Vbgb,
        "Wqb": wprep(Wq),
        "Wkb": wprep(Wk),
        "Wvb": wprep(Wv),
        "WoBb": WoBb,
    }
    qs = L // N_CORES
    return [
        dict(
            common,
            hqTb=np.ascontiguousarray(hTb[:, :, qs * c : qs * (c + 1)]),
        )
        for c in range(N_CORES)
    ]


_NC_CACHE = {}


def kernel(hidden_states, K_bg, V_bg, Wq, Wk, Wv, Wo, bo):
    if "nc" not in _NC_CACHE:
        _NC_CACHE["nc"] = build_bass()
    nc = _NC_CACHE["nc"]
    in_maps = make_in_maps(hidden_states, K_bg, V_bg, Wq, Wk, Wv, Wo, bo)
    from concourse import bass2jax

    results = bass2jax.run_bass_via_pjrt(nc, in_maps, n_cores=N_CORES)
    out = np.concatenate([results[c]["out"] for c in range(N_CORES)], axis=0)
    return out.reshape(B, L, C)


# revision 26
# speedup vs baseline: 1.1224x; 1.1224x over previous
"""CARC attention processor kernel for 8 Trainium2 NeuronCores.

Reference computation (B=1, L=4096, C=640, H=10, D=64):
    q/k/v = hidden @ Wq/Wk/Wv, split into 10 heads of 64
    k_cat = [k, 0.42*K_bg], v_cat = [v, 0.42*V_bg]   (key length 8192)
    out   = softmax(q k_cat^T / 8) v_cat, heads merged, @ Wo + bo

Sharding: queries are split 512 per core; every core computes all 10 heads
for its queries (k/v projections replicated per core).  Output is a
disjoint row-slice per core; the host concatenates.

All tensors are pre-converted to bf16 and pre-laid-out on the host, so DMA
lands directly in the SBUF layouts the matmuls consume (no staging casts).

Softmax skips max-subtraction (scores are ~N(0,1)).  The exp work is split
between ScalarE (hardware Exp LUT) and VectorE (Schraudolph fast-exp2:
one tensor_scalar computing round(a*s + b) into int16, bitcast to bf16),
alternating key tiles, so neither engine is the bottleneck.  The softmax
denominator comes from a ones-column appended to V in the probs@V matmul;
the output-projection bias is folded in as a 65th row of Wo.

Heads are processed in pairs (head A on partitions 0-63, head B on 64-127);
the two QK^T matmuls of a pair co-stream through disjoint PE row groups
via tile_position.
"""

import numpy as np

import concourse.bass as bass
import concourse.mybir as mybir
import concourse.tile as tile

F32 = mybir.dt.float32
BF16 = mybir.dt.bfloat16
I16 = mybir.dt.int16
I8 = mybir.dt.int8
E4 = mybir.dt.float8e4
E5 = mybir.dt.float8e5
AF = mybir.ActivationFunctionType
ALU = mybir.AluOpType

# Problem constants (hardcoded per contract)
B, L, C = 1, 4096, 640
H, D = 10, 64
ALPHA = 0.42
N_CORES = 8
SCALE = 1.0 / np.sqrt(D)  # 0.125
LOG2E = 1.4426950408889634
# fast-exp2 magic: i16 = rint(FEXP_A*escale*s + FEXP_B); bitcast bf16 ~ exp(escale*s)
# (DVE converts fp32->i16 round-half-even; c=-5.6 calibrated, max rel err 3.3%)
FEXP_B = 127.0 * 128.0 - 5.6
# e5m2 variant for fp8 probs: i8 = rint(4*log2(e)*escale*s + FEXP5_B); bits ~ e5m2
FEXP5_B = 15.0 * 4.0 - 1.4


class Cfg:
    def __init__(self, H=H, C=C, Lk=L, Q=L // N_CORES):
        assert C % 128 == 0 and Lk % 1024 == 0 and Q % 128 == 0 and Q <= 512
        assert H % 2 == 0
        self.H, self.C, self.Lk, self.Q = H, C, Lk, Q
        self.n_cc = C // 128      # contraction chunks for projections
        self.n_kt = Lk // 128     # key tiles per source (self / bg)
        self.n_qt = Q // 128      # query tiles of this core


def emit(nc: bass.Bass, cfg: Cfg):
    Hh, Cc, Lk, Q = cfg.H, cfg.C, cfg.Lk, cfg.Q
    n_cc, n_kt, n_qt = cfg.n_cc, cfg.n_kt, cfg.n_qt
    n_pair = Hh // 2

    hT = nc.declare_dram_parameter("hTb", [128, n_cc, Lk], BF16, isOutput=False)
    hqT = nc.declare_dram_parameter("hqTb", [128, n_cc, Q], BF16, isOutput=False)
    kbg = nc.declare_dram_parameter("Kbgb", [n_pair, 128, Lk], BF16, isOutput=False)
    vbg = nc.declare_dram_parameter(
        "Vbgb", [n_pair, 128, n_kt, 2, D + 1], BF16, isOutput=False
    )
    wq = nc.declare_dram_parameter("Wqb", [128, n_cc, Cc], BF16, isOutput=False)
    wk = nc.declare_dram_parameter("Wkb", [128, n_cc, Cc], BF16, isOutput=False)
    wv = nc.declare_dram_parameter("Wvb", [128, n_cc, Cc], BF16, isOutput=False)
    wob = nc.declare_dram_parameter("WoBb", [D + 1, Hh, Cc], BF16, isOutput=False)
    out = nc.declare_dram_parameter("out", [Q, Cc], F32, isOutput=True)

    with tile.TileContext(nc) as tc:
        with (
            tc.tile_pool(name="singles", bufs=1) as singles,
            tc.tile_pool(name="kv", bufs=2) as kv,
            tc.tile_pool(name="probs", bufs=4) as probs_pool,
            tc.tile_pool(name="outsb", bufs=2) as outsb_pool,
            tc.tile_pool(name="fin", bufs=2) as fin_pool,
            tc.tile_pool(name="ps_a", bufs=2, space="PSUM") as ps_a,
            tc.tile_pool(name="ps_sc", bufs=2, space="PSUM") as ps_sc,
            tc.tile_pool(name="ps_ctx", bufs=1, space="PSUM") as ps_ctx,
        ):
            # ---- persistent SBUF tensors, DMA'd directly in final layout ----
            hq_bf = singles.tile([128, n_cc, Q], BF16, tag="hq_bf")
            wq_bf = singles.tile([128, n_cc, Cc], BF16, tag="wq_bf")
            wk_bf = singles.tile([128, n_cc, Cc], BF16, tag="wk_bf")
            wv_bf = singles.tile([128, n_cc, Cc], BF16, tag="wv_bf")
            wob_bf = singles.tile([D + 1, Hh, Cc], BF16, tag="wob_bf")
            hT_bf = singles.tile([128, n_cc, Lk], BF16, tag="hT_bf")
            qT2_all = singles.tile([128, n_pair, Q], BF16, tag="qT2_all")
            ctxT_all = singles.tile([D + 1, Hh, Q], BF16, tag="ctxT_all")
            ones64 = singles.tile([D + 1, D], F32, tag="ones64")
            nc.vector.memset(ones64, 1.0)
            for h in range(Hh):
                nc.vector.memset(ctxT_all[D : D + 1, h, :], 1.0)

            nc.sync.dma_start(out=hq_bf, in_=hqT[:, :, :])
            nc.sync.dma_start(out=wq_bf, in_=wq[:, :, :])
            nc.sync.dma_start(out=wk_bf, in_=wk[:, :, :])
            nc.sync.dma_start(out=wv_bf, in_=wv[:, :, :])
            nc.sync.dma_start(out=wob_bf, in_=wob[:, :, :])
            # hidden transposed, split into 4 DMAs so the first k-proj block
            # can start earlier
            qtr = Lk // 4
            for qi in range(4):
                nc.sync.dma_start(
                    out=hT_bf[:, :, qtr * qi : qtr * (qi + 1)],
                    in_=hT[:, :, qtr * qi : qtr * (qi + 1)],
                )

            # ---- q projections, head pairs packed on partitions ----
            for p in range(n_pair):
                ps = ps_a.tile([128, 512], F32, tag="ps_a", name=f"qps{p}")
                for i in range(n_cc):
                    nc.tensor.matmul(
                        ps[:, 0:Q],
                        lhsT=wq_bf[:, i, 128 * p : 128 * (p + 1)],
                        rhs=hq_bf[:, i, :],
                        start=(i == 0),
                        stop=(i == n_cc - 1),
                    )
                nc.scalar.copy(out=qT2_all[:, p, :], in_=ps[:, 0:Q])

            # ---- deferred per-pair normalization ----
            norm_queue = []

            def emit_normalize():
                while norm_queue:
                    pn, unn = norm_queue.pop(0)
                    for hi in range(2):
                        h = 2 * pn + hi
                        fin = fin_pool.tile(
                            [D + 1, Q], F32, tag="fin", name=f"fin{h}"
                        )
                        bc = ps_a.tile([128, 512], F32, tag="ps_a", name=f"bc{h}")
                        nc.tensor.matmul(
                            bc[0:D, 0:Q],
                            lhsT=ones64[D : D + 1, :],
                            rhs=unn[D : D + 1, hi, :],
                            start=True,
                            stop=True,
                            tile_position=(D, 0),
                        )
                        nc.vector.reciprocal(fin[0:D, :], bc[0:D, 0:Q])
                        nc.vector.tensor_mul(
                            ctxT_all[0:D, h, :], unn[0:D, hi, :], fin[0:D, :]
                        )

            # ---- per head-pair: project k/v, load bg kv, attention ----
            for p in range(n_pair):
                kT2 = kv.tile([128, Lk], BF16, tag="kT")
                v2 = kv.tile([128, n_kt, 2, D + 1], BF16, tag="v")
                kbg2 = kv.tile([128, Lk], BF16, tag="kbg")
                vbg2 = kv.tile([128, n_kt, 2, D + 1], BF16, tag="vbg")

                nc.sync.dma_start(out=kbg2, in_=kbg[p, :, :])
                nc.sync.dma_start(out=vbg2, in_=vbg[p, :, :, :, :])

                # kT2 = (hidden @ Wk_pair)^T, head A on partitions 0-63
                for t in range(Lk // 512):
                    ps = ps_a.tile([128, 512], F32, tag="ps_a", name=f"kps{p}{t}")
                    for i in range(n_cc):
                        nc.tensor.matmul(
                            ps,
                            lhsT=wk_bf[:, i, 128 * p : 128 * (p + 1)],
                            rhs=hT_bf[:, i, 512 * t : 512 * (t + 1)],
                            start=(i == 0),
                            stop=(i == n_cc - 1),
                        )
                    if t % 2 == 0:
                        nc.scalar.copy(out=kT2[:, 512 * t : 512 * (t + 1)], in_=ps)
                    else:
                        nc.vector.tensor_copy(
                            out=kT2[:, 512 * t : 512 * (t + 1)], in_=ps
                        )
                # v natural [keys, D] for both heads (+ones cols); 4 key
                # tiles packed per PSUM bank so each PSUM->SBUF copy moves
                # 4 tiles at once
                for g in range(n_kt // 4):
                    ps = ps_a.tile([128, 4, 128], F32, tag="ps_a", name=f"vps{p}{g}")
                    for j in range(4):
                        kt = 4 * g + j
                        for i in range(n_cc):
                            nc.tensor.matmul(
                                ps[:, j, :],
                                lhsT=hT_bf[:, i, 128 * kt : 128 * (kt + 1)],
                                rhs=wv_bf[:, i, 128 * p : 128 * (p + 1)],
                                start=(i == 0),
                                stop=(i == n_cc - 1),
                            )
                    for hi in range(2):
                        dst = v2[:, 4 * g : 4 * (g + 1), hi, 0:D]
                        srcp = ps[:, :, D * hi : D * (hi + 1)]
                        if g % 2 == 0:
                            nc.scalar.copy(out=dst, in_=srcp)
                        else:
                            nc.vector.tensor_copy(out=dst, in_=srcp)
                for hi in range(2):
                    nc.vector.memset(v2[:, :, hi, D : D + 1], 1.0)

                # normalize the PREVIOUS pair now that this pair's projection
                # matmuls are already queued ahead of it
                emit_normalize()

                # ---- attention for the pair ----
                # ctx accumulators: head A in PSUM bank 0, head B in bank 1.
                # Software-pipelined: scores+exp for step i are issued before
                # the ctx matmuls of step i-2, so the PE never waits on exp.
                ctx2 = ps_ctx.tile([D + 1, 2, 512], F32, tag="ctx", name=f"ctx{p}")
                seq = [(src, kt) for src in range(2) for kt in range(n_kt)]

                def issue_front(idx):
                    src, kt = seq[idx]
                    kk = kT2 if src == 0 else kbg2
                    e_scale = SCALE if src == 0 else SCALE * ALPHA
                    scs = ps_sc.tile(
                        [128, 2, Q], F32, tag="sc", name=f"sc{p}{src}{kt}"
                    )
                    for hi in range(2):
                        nc.tensor.matmul(
                            scs[:, hi, :],
                            lhsT=kk[
                                D * hi : D * (hi + 1),
                                128 * kt : 128 * (kt + 1),
                            ],
                            rhs=qT2_all[D * hi : D * (hi + 1), p, :],
                            start=True,
                            stop=True,
                            tile_position=(D * hi, 0),
                        )
                    # exp alternates ScalarE (LUT) / VectorE (fast-exp2)
                    if idx % 2 == 0:
                        pr = probs_pool.tile(
                            [128, 2, Q], BF16, tag="pr", name=f"pr{p}{src}{kt}"
                        )
                        nc.scalar.activation(pr, scs, AF.Exp, scale=e_scale)
                        pr_bf = pr
                    else:
                        pri = probs_pool.tile(
                            [128, 2, Q], I16, tag="pri", name=f"pri{p}{src}{kt}"
                        )
                        nc.vector.tensor_scalar(
                            pri,
                            scs,
                            128.0 * LOG2E * e_scale,
                            FEXP_B,
                            ALU.mult,
                            ALU.add,
                        )
                        pr_bf = pri.bitcast(BF16)
                    return (v2 if src == 0 else vbg2), kt, pr_bf, idx == 0

                def issue_ctx(st, last):
                    vv, kt, pr_bf, first = st
                    for hi in range(2):
                        nc.tensor.matmul(
                            ctx2[:, hi, 0:Q],
                            lhsT=vv[:, kt, hi, :],
                            rhs=pr_bf[:, hi, :],
                            start=first,
                            stop=last,
                        )

                # 2-kt batches: [sc,sc,sc,sc] then [ctx,ctx,ctx,ctx] so
                # same-row-group LDWEIGHTS hide in the background weight
                # buffer and only two row-group boundaries are exposed per
                # batch instead of four.
                DEPTH = 4
                pipe = []
                for idx in range(0, len(seq), 2):
                    pipe.append(issue_front(idx))
                    pipe.append(issue_front(idx + 1))
                    while len(pipe) > DEPTH:
                        issue_ctx(pipe.pop(0), False)
                for i, st in enumerate(pipe):
                    issue_ctx(st, i == len(pipe) - 1)
                # Free the ctx PSUM banks immediately with one wide copy to
                # SBUF; the normalize itself is emitted deferred (after the
                # next pair's projections) so its PSUM-ring slots don't gate
                # the next pair's projection matmuls.
                unn = fin_pool.tile([D + 1, 2, Q], F32, tag="unn", name=f"unn{p}")
                nc.scalar.copy(out=unn, in_=ctx2)
                norm_queue.append((p, unn))
                if p == n_pair - 1:
                    emit_normalize()

            # ---- output projection: out[qt] = sum_h ctxT_h^T @ WoB_h ----
            for qt in range(n_qt):
                o_sb = outsb_pool.tile([128, Cc], F32, tag="o_sb")
                for n0 in range(0, Cc, 512):
                    nw = min(512, Cc - n0)
                    ps = ps_sc.tile([128, 2, Q], F32, tag="sc", name=f"ops{qt}{n0}")
                    for h in range(Hh):
                        nc.tensor.matmul(
                            ps[:, 0, 0:nw],
                            lhsT=ctxT_all[:, h, 128 * qt : 128 * (qt + 1)],
                            rhs=wob_bf[:, h, n0 : n0 + nw],
                            start=(h == 0),
                            stop=(h == Hh - 1),
                        )
                    if n0 == 0:
                        nc.scalar.copy(out=o_sb[:, n0 : n0 + nw], in_=ps[:, 0, 0:nw])
                    else:
                        nc.vector.tensor_copy(
                            out=o_sb[:, n0 : n0 + nw], in_=ps[:, 0, 0:nw]
                        )
                nc.sync.dma_start(out=out[128 * qt : 128 * (qt + 1), :], in_=o_sb)
    return nc


def split_waits(nc, limit=1):
    """This container's walrus rejects >limit sync waits per instruction;
    hoist excess waits onto standalone EventSemaphore instructions."""
    cnt = 0
    for f in nc.m.functions:
        for bb in f.blocks:
            fixed = []
            for inst in bb.instructions:
                si = inst.sync_info
                if si is not None and len(si.on_wait) > limit:
                    waits = list(si.on_wait)
                    extra, keep = waits[:-limit], waits[-limit:]
                    for w in extra:
                        cnt += 1
                        ev = mybir.InstEventSemaphore(
                            name=f"I-waitsplit-{cnt}", ins=[], outs=[]
                        )
                        ev.engine = inst.engine
                        ev.sync_info = mybir.SyncInfo(on_wait=[w], on_update=[])
                        nc.register_instruction(ev)
                        fixed.append(ev)
                    si.on_wait = keep
                fixed.append(inst)
            bb.instructions[:] = fixed
    return cnt


def build_bass(cfg: Cfg | None = None):
    cfg = cfg or Cfg()
    nc = bass.Bass()
    emit(nc, cfg)
    split_waits(nc)
    return nc


def make_in_maps(hidden_states, K_bg, V_bg, Wq, Wk, Wv, Wo, bo):
    import ml_dtypes

    bf = ml_dtypes.bfloat16
    n_cc, n_kt, n_pair = C // 128, L // 128, H // 2

    hT = np.asarray(hidden_states, np.float32)[0].T  # [C, L]
    hTb = np.ascontiguousarray(
        hT.reshape(n_cc, 128, L).transpose(1, 0, 2)
    ).astype(bf)

    def wprep(w):
        return np.ascontiguousarray(
            np.asarray(w, np.float32).reshape(n_cc, 128, C).transpose(1, 0, 2)
        ).astype(bf)

    WoB = np.zeros((H, D + 1, C), np.float32)
    WoB[:, :D, :] = np.asarray(Wo, np.float32).reshape(H, D, C)
    WoB[0, D, :] = np.asarray(bo, np.float32)
    WoBb = np.ascontiguousarray(WoB.transpose(1, 0, 2)).astype(bf)

    Kbgb = np.ascontiguousarray(
        np.asarray(K_bg, np.float32).transpose(0, 2, 1).reshape(n_pair, 128, L)
    ).astype(bf)
    Vbgb = np.ones((n_pair, 128, n_kt, 2, D + 1), np.float32)
    Vbgb[:, :, :, :, :D] = (
        (ALPHA * np.asarray(V_bg, np.float32))
        .reshape(n_pair, 2, n_kt, 128, D)
        .transpose(0, 3, 2, 1, 4)
    )
    Vbgb = Vbgb.astype(bf)

    common = {
        "hTb": hTb,
        "Kbgb": Kbgb,
        "Vbgb": Vbgb,
        "Wqb": wprep(Wq),
        "Wkb": wprep(Wk),
        "Wvb": wprep(Wv),
        "WoBb": WoBb,
    }
    qs = L // N_CORES
    return [
        dict(
            common,
            hqTb=np.ascontiguousarray(hTb[:, :, qs * c : qs * (c + 1)]),
        )
        for c in range(N_CORES)
    ]


_NC_CACHE = {}


def kernel(hidden_states, K_bg, V_bg, Wq, Wk, Wv, Wo, bo):
    if "nc" not in _NC_CACHE:
        _NC_CACHE["nc"] = build_bass()
    nc = _NC_CACHE["nc"]
    in_maps = make_in_maps(hidden_states, K_bg, V_bg, Wq, Wk, Wv, Wo, bo)
    from concourse import bass2jax

    results = bass2jax.run_bass_via_pjrt(nc, in_maps, n_cores=N_CORES)
    out = np.concatenate([results[c]["out"] for c in range(N_CORES)], axis=0)
    return out.reshape(B, L, C)
